# revision 16
# baseline (speedup 1.0000x reference)
"""Trainium2 Bass kernel for nn_Column_82136954569126 (topk_masking).

Computes: out = einsum('tchw,kchw->tk', rec_field, weight) -> threshold ->
spike stats -> k-WTA top-16 winner mask -> masked spike wave (T, K, 1, 1).

Sharding: weight (and the K output-feature dim) is split across 8 cores
(tensor parallel over out_channels). Each core computes its (T=64, 256)
slice of the projection with 512 accumulating fp32 matmuls (contraction
chunked by 128 on the partition dim), derives per-feature ranking scores,
AllGathers the 2048 scores, selects the global top-16 by rank, and writes
its masked spike slice. Host only reshapes/shards/gathers.
"""

import os
import numpy as np

import concourse.bacc as bacc
import concourse.mybir as mybir
import concourse.tile as tile
from concourse import bass_utils

N_CORES = 8
T = 64              # timesteps
K = 2048            # total output features
KL = K // N_CORES   # features per core (256)
C = 65536           # contraction size (1*256*256)
P = 128             # SBUF partitions
NCHUNK = C // P     # 512 contraction chunks
THRESH = 16384.0
KWTA = 16
VBIAS = 2097152.0   # constant >> max(n*first_pot); ranking-equivalent to ref's v
WB = 8              # weight chunks batched per DMA (1 MiB)
NWT = NCHUNK // WB  # weight DMA blocks (64)
NSLC = 16           # rec DMA blocks (1 MiB each)
SCH = NCHUNK // NSLC  # contraction chunks per rec block (32)

_nc_cache = None
LAST_RESULT = None


def _build():
    nc = bacc.Bacc("TRN2", target_bir_lowering=False, debug=False,
                   num_devices=N_CORES)
    f32 = mybir.dt.float32

    # Device-tiled layouts (host prepares; see kernel() for the packing).
    # Each DMA block is a fully contiguous DRAM region:
    #  rec_dev[s*128+p, c*T + t] = rec[t, (s*SCH+c)*128 + p]
    #  w_dev[ab*128+p, b*KL + k] = W[k0 + k, (ab*WB+b)*128 + p]
    rec_in = nc.dram_tensor("rec_dev", [NSLC * P, SCH * T], f32,
                            kind="ExternalInput").ap()
    w_in = nc.dram_tensor("w_dev", [NWT * P, WB * KL], f32,
                          kind="ExternalInput").ap()
    ident_in = nc.dram_tensor("ident", [P, P], f32, kind="ExternalInput").ap()
    iota_in = nc.dram_tensor("iota_t", [1, T], f32, kind="ExternalInput").ap()
    out_spk = nc.dram_tensor("out_spk", [KL, T], f32, kind="ExternalOutput").ap()

    with tile.TileContext(nc) as tc:
        with tc.tile_pool(name="rec", bufs=6) as rec_pool, \
             tc.tile_pool(name="wt", bufs=8) as wt_pool, \
             tc.tile_pool(name="small", bufs=1) as small, \
             tc.tile_pool(name="ps", bufs=1, space="PSUM") as ps, \
             tc.tile_pool(name="pst", bufs=2, space="PSUM") as pst, \
             tc.tile_pool(name="dram", bufs=1, space="DRAM") as dram:

            ident = small.tile([P, P], f32)
            nc.sync.dma_start(ident[:], ident_in[:])
            iota_t = small.tile([P, T], f32)
            nc.sync.dma_start(iota_t[:], iota_in.broadcast_to([P, T]))

            # ---- the big matmul: acc[t, k] += rec_chunk.T @ w_chunk
            # rec slices (16 x 1 MiB, persistent) are interleaved into the
            # weight stream so the PE starts after ~2 MiB of DMA, not 16.8.
            rec_sl = []
            acc = ps.tile([T, KL], f32)
            grp = NWT // NSLC
            for ab in range(NWT):
                # prefetch rec slice s=ab//grp+1 half a group early (s=0 at ab=0)
                if (ab == 0 or ab % grp == grp // 2) and len(rec_sl) < NSLC:
                    s = len(rec_sl)
                    r = rec_pool.tile([P, SCH * T], f32, tag="recs")
                    nc.sync.dma_start(r[:], rec_in[s * P:(s + 1) * P, :])
                    rec_sl.append(r)
                w_sb = wt_pool.tile([P, WB * KL], f32, tag="w")
                nc.sync.dma_start(w_sb[:], w_in[ab * P:(ab + 1) * P, :])
                for b in range(WB):
                    a = ab * WB + b
                    r = rec_sl[a // SCH]
                    ai = a % SCH
                    nc.tensor.matmul(acc[:],
                                     r[:, ai * T:(ai + 1) * T],
                                     w_sb[:, b * KL:(b + 1) * KL],
                                     start=(a == 0), stop=(a == NCHUNK - 1))

            # ---- transpose acc (64, 256) -> two (128, 64) halves
            mm_sb = small.tile([T, KL], f32)
            nc.vector.tensor_copy(mm_sb[:], acc[:])
            outT = small.tile([P, 2 * T], f32)   # [k_local(128) , half*64 + t]
            for h in range(2):
                accT = pst.tile([P, T], f32, tag="accT")
                nc.tensor.transpose(accT[:], mm_sb[:, h * P:(h + 1) * P],
                                    ident[:T, :T])
                nc.vector.tensor_copy(outT[:, h * T:(h + 1) * T], accT[:])

            # ---- per-feature stats (k on partitions, t on free dim)
            spikes = small.tile([P, 2 * T], f32)
            score = small.tile([P, 2], f32)
            n_t = small.tile([P, 2], f32)
            scratch = small.tile([P, T], f32)
            for h in range(2):
                sl = slice(h * T, (h + 1) * T)
                nh = n_t[:, h:h + 1]
                # spikes = out > thresh, n = sum(spikes)  (fused accumulate)
                nc.vector.tensor_scalar(spikes[:, sl], outT[:, sl], THRESH, 0.0,
                                        mybir.AluOpType.is_gt,
                                        mybir.AluOpType.add, accum_out=nh)
                # first-spike index = T - n ; one-hot match against iota
                fi = small.tile([P, 1], f32, tag=f"fi{h}")
                nc.vector.tensor_scalar(fi[:], nh, -1.0, float(T),
                                        mybir.AluOpType.mult, mybir.AluOpType.add)
                isf = small.tile([P, T], f32, tag=f"isf{h}")
                nc.vector.tensor_scalar(isf[:], iota_t[:, :T], fi[:], None,
                                        mybir.AluOpType.is_equal)
                # one_hot &= spike ; first_pot = sum(out * one_hot)
                nc.vector.scalar_tensor_tensor(isf[:], outT[:, sl], THRESH, isf[:],
                                               mybir.AluOpType.is_gt,
                                               mybir.AluOpType.mult)
                fp = small.tile([P, 1], f32, tag=f"fp{h}")
                nc.vector.scalar_tensor_tensor(scratch[:], outT[:, sl], 1.0, isf[:],
                                               mybir.AluOpType.mult,
                                               mybir.AluOpType.mult, accum_out=fp[:])
                # score = (first_pot + VBIAS) * n
                nc.vector.tensor_scalar(score[:, h:h + 1], fp[:], VBIAS, nh,
                                        mybir.AluOpType.add, mybir.AluOpType.mult)

            # ---- AllGather the 256 local scores -> 2048 global scores
            # pack scores contiguously: transpose [128,2] -> [2,128]
            sT_ps = pst.tile([2, P], f32, tag="sT")
            nc.tensor.transpose(sT_ps[:], score[:], ident[:])
            sT = small.tile([2, P], f32)
            nc.vector.tensor_copy(sT[:], sT_ps[:])
            s_in = dram.tile([2, P], f32)
            s_out = dram.tile([1, K], f32)
            nc.sync.dma_start(s_in[:], sT[:])
            nc.gpsimd.collective_compute(
                "AllGather", mybir.AluOpType.bypass,
                replica_groups=[list(range(N_CORES))],
                ins=[s_in.opt()], outs=[s_out.opt()],
            )

            # ---- rank each local feature among all 2048 scores
            g = small.tile([P, K], f32)
            nc.sync.dma_start(g[:], s_out[:].broadcast_to([P, K]))
            masked = small.tile([P, 2 * T], f32)
            cmp = small.tile([P, K], f32)
            for h in range(2):
                sh = score[:, h:h + 1]
                # rank = #{j : s_all[j] > score_k}  (fused accumulate)
                rank = small.tile([P, 1], f32, tag=f"rank{h}")
                nc.vector.tensor_scalar(cmp[:], g[:], sh, 0.0,
                                        mybir.AluOpType.is_gt,
                                        mybir.AluOpType.add, accum_out=rank[:])
                # coef = (rank < KWTA) & (score > 0)
                ltm = small.tile([P, 1], f32, tag=f"ltm{h}")
                nc.vector.tensor_scalar(ltm[:], rank[:], float(KWTA), None,
                                        mybir.AluOpType.is_lt)
                coef = small.tile([P, 1], f32, tag=f"coef{h}")
                nc.vector.scalar_tensor_tensor(coef[:], sh, 0.0, ltm[:],
                                               mybir.AluOpType.is_gt,
                                               mybir.AluOpType.mult)
                sl = slice(h * T, (h + 1) * T)
                nc.vector.tensor_scalar(masked[:, sl], spikes[:, sl], coef[:],
                                        None, mybir.AluOpType.mult)
                nc.sync.dma_start(out_spk[h * P:(h + 1) * P, :], masked[:, sl])

    nc.compile()
    return nc


def kernel(rec_field: np.ndarray, weight: np.ndarray) -> np.ndarray:
    global _nc_cache, LAST_RESULT
    rec = np.ascontiguousarray(rec_field, dtype=np.float32).reshape(T, C)
    w = np.ascontiguousarray(weight, dtype=np.float32).reshape(K, C)

    # host-side re-tiling (sharding layout prep); every DMA block contiguous
    rec_dev = np.ascontiguousarray(
        rec.reshape(T, NSLC, SCH, P).transpose(1, 3, 2, 0).reshape(NSLC * P, SCH * T))
    ident = np.eye(P, dtype=np.float32)
    iota_t = np.arange(T, dtype=np.float32)[None, :]

    in_maps = []
    for c in range(N_CORES):
        wsh = w[c * KL:(c + 1) * KL]  # (256, 65536)
        w_dev = np.ascontiguousarray(
            wsh.reshape(KL, NWT, WB, P).transpose(1, 3, 2, 0).reshape(NWT * P, WB * KL))
        in_maps.append({
            "rec_dev": rec_dev,
            "w_dev": w_dev,
            "ident": ident,
            "iota_t": iota_t,
        })

    if _nc_cache is None:
        _nc_cache = _build()
    res = bass_utils.run_bass_kernel_spmd(
        _nc_cache, in_maps, core_ids=list(range(N_CORES)),
        trace=bool(os.environ.get("KERNEL_TRACE")),
    )
    LAST_RESULT = res

    shards = [res.results[c]["out_spk"] for c in range(N_CORES)]  # (256, 64) each
    full = np.concatenate(shards, axis=0)          # (2048, 64)
    out = full.T.astype(np.float32)                # (64, 2048)
    return np.ascontiguousarray(out).reshape(T, K, 1, 1)


# revision 18
# speedup vs baseline: 1.1125x; 1.1125x over previous
"""Trainium2 Bass kernel for nn_Column_82136954569126 (topk_masking).

Computes: out = einsum('tchw,kchw->tk', rec_field, weight) -> threshold ->
spike stats -> k-WTA top-16 winner mask -> masked spike wave (T, K, 1, 1).

Sharding (8 cores, 2D): out-features K=2048 split into 4 groups x 512;
contraction C=65536 split into 2 halves x 32768. Core c handles k-group
g=c%4 and c-half m=c//4: it streams its 64 MiB weight quadrant through
512-wide fp32 accumulating matmuls (contraction chunked by 128 on the
partition dim, even/odd chunks into the two PSUM partition halves so
LDWEIGHTS can overlap), PE-transposes the partial (64,512) to
feature-major, pairwise-ReduceScatters with its c-half partner so each
core ends up with the full projection for its own 256 features, derives
per-feature ranking scores, AllGathers the 2048 scores, selects the
global top-16 by rank, and writes its masked spike slice. The host only
re-tiles/shards inputs and re-assembles the output shards.
"""

import os
import numpy as np

import concourse.bacc as bacc
import concourse.mybir as mybir
import concourse.tile as tile
from concourse import bass_utils

N_CORES = 8
T = 64                 # timesteps
K = 2048               # total output features
P = 128                # SBUF partitions
C = 65536              # full contraction size (1*256*256)
KG = 4                 # k groups
CS = 2                 # contraction split
KW = K // KG           # features per matmul (512)
KL = KW // CS          # features per core for stats/output (256)
CH = C // CS           # contraction per core (32768)
NCHUNK = CH // P       # contraction chunks per core (256)
THRESH = 16384.0
KWTA = 16
VBIAS = 2097152.0      # constant >> max(n*first_pot); ranking-equivalent to ref's v
WB = 4                 # chunks per weight DMA block (1 MiB)
NWT = NCHUNK // WB     # weight DMA blocks (64)
NSLC = 8               # rec DMA blocks (1 MiB each)
SCH = NCHUNK // NSLC   # chunks per rec block (32)

_nc_cache = None
LAST_RESULT = None


def _build():
    nc = bacc.Bacc("TRN2", target_bir_lowering=False, debug=False,
                   num_devices=N_CORES)
    f32 = mybir.dt.float32

    # Device-tiled layouts (host prepares; every DMA block is contiguous):
    #  rec_dev[s*128+p, ci*T+t] = rec[t, m*CH + (s*SCH+ci)*128 + p]
    #  w_dev[ab*128+p, b*KW+k]  = W[g*KW + k, m*CH + (ab*WB+b)*128 + p]
    rec_in = nc.dram_tensor("rec_dev", [NSLC * P, SCH * T], f32,
                            kind="ExternalInput").ap()
    w_in = nc.dram_tensor("w_dev", [NWT * P, WB * KW], f32,
                          kind="ExternalInput").ap()
    ident_in = nc.dram_tensor("ident", [P, P], f32, kind="ExternalInput").ap()
    iota_in = nc.dram_tensor("iota_t", [1, T], f32, kind="ExternalInput").ap()
    out_spk = nc.dram_tensor("out_spk", [KL, T], f32, kind="ExternalOutput").ap()

    with tile.TileContext(nc) as tc:
        with tc.tile_pool(name="rec", bufs=5) as rec_pool, \
             tc.tile_pool(name="wt", bufs=8) as wt_pool, \
             tc.tile_pool(name="small", bufs=1) as small, \
             tc.tile_pool(name="ps", bufs=1, space="PSUM") as ps, \
             tc.tile_pool(name="pst", bufs=2, space="PSUM") as pst, \
             tc.tile_pool(name="dram", bufs=1, space="DRAM") as dram:

            ident = small.tile([P, P], f32)
            nc.sync.dma_start(ident[:], ident_in[:])
            iota_t = small.tile([P, T], f32)
            nc.sync.dma_start(iota_t[:], iota_in.broadcast_to([P, T]))

            # ---- the big matmul: acc[t, k] += rec_chunk.T @ w_chunk
            # even chunks -> PSUM partitions 0..63, odd -> 64..127 so each
            # chunk's LDWEIGHTS targets the idle column half of the PE array.
            rec_sl = []
            acc = ps.tile([P, KW], f32)
            grp = NWT // NSLC
            for ab in range(NWT):
                if (ab == 0 or ab % grp == grp // 2) and len(rec_sl) < NSLC:
                    s = len(rec_sl)
                    r = rec_pool.tile([P, SCH * T], f32, tag="recs")
                    nc.sync.dma_start(r[:], rec_in[s * P:(s + 1) * P, :])
                    rec_sl.append(r)
                w_sb = wt_pool.tile([P, WB * KW], f32, tag="w")
                nc.sync.dma_start(w_sb[:], w_in[ab * P:(ab + 1) * P, :])
                for b in range(WB):
                    a = ab * WB + b
                    r = rec_sl[a // SCH]
                    ai = a % SCH
                    hrow = (a & 1) * T
                    nc.tensor.matmul(acc[hrow:hrow + T, :],
                                     r[:, ai * T:(ai + 1) * T],
                                     w_sb[:, b * KW:(b + 1) * KW],
                                     start=(a < 2), stop=(a >= NCHUNK - 2))

            # ---- combine halves, transpose to feature-major [512, 64]
            mm_sb = small.tile([T, KW], f32)
            nc.vector.tensor_copy(mm_sb[:], acc[T:2 * T, :])
            nc.vector.tensor_tensor(mm_sb[:], acc[0:T, :], mm_sb[:],
                                    mybir.AluOpType.add)
            outTfull = small.tile([P, 4 * T], f32)
            for q in range(4):
                tq = pst.tile([P, T], f32, tag="tq")
                nc.tensor.transpose(tq[:], mm_sb[:, q * P:(q + 1) * P],
                                    ident[:T, :T])
                nc.vector.tensor_copy(outTfull[:, q * T:(q + 1) * T], tq[:])

            # ---- pairwise ReduceScatter over the two c-halves: each core
            # receives the complete projection for its own 256 features.
            rs_in = dram.tile([KW, T], f32)
            rs_out = dram.tile([KL, T], f32)
            for q in range(4):
                nc.sync.dma_start(rs_in[q * P:(q + 1) * P, :],
                                  outTfull[:, q * T:(q + 1) * T])
            nc.gpsimd.collective_compute(
                "ReduceScatter", mybir.AluOpType.add,
                replica_groups=[[0, 4], [1, 5], [2, 6], [3, 7]],
                ins=[rs_in.opt()], outs=[rs_out.opt()],
            )
            outT = small.tile([P, 2 * T], f32)   # [k_local(128), half*64 + t]
            for h in range(2):
                nc.sync.dma_start(outT[:, h * T:(h + 1) * T],
                                  rs_out[h * P:(h + 1) * P, :])

            # ---- per-feature stats (k on partitions, t on free dim)
            spikes = small.tile([P, 2 * T], f32)
            score = small.tile([P, 2], f32)
            n_t = small.tile([P, 2], f32)
            scratch = small.tile([P, T], f32)
            for h in range(2):
                sl = slice(h * T, (h + 1) * T)
                nh = n_t[:, h:h + 1]
                # spikes = out > thresh, n = sum(spikes)  (fused accumulate)
                nc.vector.tensor_scalar(spikes[:, sl], outT[:, sl], THRESH, 0.0,
                                        mybir.AluOpType.is_gt,
                                        mybir.AluOpType.add, accum_out=nh)
                # first-spike index = T - n ; one-hot match against iota
                fi = small.tile([P, 1], f32, tag=f"fi{h}")
                nc.vector.tensor_scalar(fi[:], nh, -1.0, float(T),
                                        mybir.AluOpType.mult, mybir.AluOpType.add)
                isf = small.tile([P, T], f32, tag=f"isf{h}")
                nc.vector.tensor_scalar(isf[:], iota_t[:, :T], fi[:], None,
                                        mybir.AluOpType.is_equal)
                # one_hot &= spike ; first_pot = sum(out * one_hot)
                nc.vector.scalar_tensor_tensor(isf[:], outT[:, sl], THRESH, isf[:],
                                               mybir.AluOpType.is_gt,
                                               mybir.AluOpType.mult)
                fp = small.tile([P, 1], f32, tag=f"fp{h}")
                nc.vector.scalar_tensor_tensor(scratch[:], outT[:, sl], 1.0, isf[:],
                                               mybir.AluOpType.mult,
                                               mybir.AluOpType.mult, accum_out=fp[:])
                # score = (first_pot + VBIAS) * n
                nc.vector.tensor_scalar(score[:, h:h + 1], fp[:], VBIAS, nh,
                                        mybir.AluOpType.add, mybir.AluOpType.mult)

            # ---- AllGather the 256 local scores -> 2048 global scores
            # (pack scores contiguously: transpose [128,2] -> [2,128])
            sT_ps = pst.tile([2, P], f32, tag="sT")
            nc.tensor.transpose(sT_ps[:], score[:], ident[:])
            sT = small.tile([2, P], f32)
            nc.vector.tensor_copy(sT[:], sT_ps[:])
            s_in = dram.tile([2, P], f32)
            s_out = dram.tile([1, K], f32)
            nc.sync.dma_start(s_in[:], sT[:])
            nc.gpsimd.collective_compute(
                "AllGather", mybir.AluOpType.bypass,
                replica_groups=[list(range(N_CORES))],
                ins=[s_in.opt()], outs=[s_out.opt()],
            )

            # ---- rank each local feature among all 2048 scores
            g = small.tile([P, K], f32)
            nc.sync.dma_start(g[:], s_out[:].broadcast_to([P, K]))
            masked = small.tile([P, 2 * T], f32)
            cmp = small.tile([P, K], f32)
            for h in range(2):
                sh = score[:, h:h + 1]
                # rank = #{j : s_all[j] > score_k}  (fused accumulate)
                rank = small.tile([P, 1], f32, tag=f"rank{h}")
                nc.vector.tensor_scalar(cmp[:], g[:], sh, 0.0,
                                        mybir.AluOpType.is_gt,
                                        mybir.AluOpType.add, accum_out=rank[:])
                # coef = (rank < KWTA) & (score > 0)
                ltm = small.tile([P, 1], f32, tag=f"ltm{h}")
                nc.vector.tensor_scalar(ltm[:], rank[:], float(KWTA), None,
                                        mybir.AluOpType.is_lt)
                coef = small.tile([P, 1], f32, tag=f"coef{h}")
                nc.vector.scalar_tensor_tensor(coef[:], sh, 0.0, ltm[:],
                                               mybir.AluOpType.is_gt,
                                               mybir.AluOpType.mult)
                sl = slice(h * T, (h + 1) * T)
                nc.vector.tensor_scalar(masked[:, sl], spikes[:, sl], coef[:],
                                        None, mybir.AluOpType.mult)
                nc.sync.dma_start(out_spk[h * P:(h + 1) * P, :], masked[:, sl])

    nc.compile()
    return nc


def kernel(rec_field: np.ndarray, weight: np.ndarray) -> np.ndarray:
    global _nc_cache, LAST_RESULT
    rec = np.ascontiguousarray(rec_field, dtype=np.float32).reshape(T, C)
    w = np.ascontiguousarray(weight, dtype=np.float32).reshape(K, C)

    # host-side re-tiling (sharding layout prep); every DMA block contiguous
    ident = np.eye(P, dtype=np.float32)
    iota_t = np.arange(T, dtype=np.float32)[None, :]

    in_maps = []
    for c in range(N_CORES):
        gk, m = c % KG, c // KG
        rec_m = rec[:, m * CH:(m + 1) * CH]                 # (64, 32768)
        rec_dev = np.ascontiguousarray(
            rec_m.reshape(T, NSLC, SCH, P).transpose(1, 3, 2, 0)
            .reshape(NSLC * P, SCH * T))
        wsh = w[gk * KW:(gk + 1) * KW, m * CH:(m + 1) * CH]  # (512, 32768)
        w_dev = np.ascontiguousarray(
            wsh.reshape(KW, NWT, WB, P).transpose(1, 3, 2, 0)
            .reshape(NWT * P, WB * KW))
        in_maps.append({
            "rec_dev": rec_dev,
            "w_dev": w_dev,
            "ident": ident,
            "iota_t": iota_t,
        })

    if _nc_cache is None:
        _nc_cache = _build()
    res = bass_utils.run_bass_kernel_spmd(
        _nc_cache, in_maps, core_ids=list(range(N_CORES)),
        trace=bool(os.environ.get("KERNEL_TRACE")),
    )
    LAST_RESULT = res

    full = np.empty((K, T), dtype=np.float32)
    for c in range(N_CORES):
        gk, m = c % KG, c // KG
        k0 = gk * KW + m * KL
        full[k0:k0 + KL] = res.results[c]["out_spk"]
    out = full.T.astype(np.float32)                # (64, 2048)
    return np.ascontiguousarray(out).reshape(T, K, 1, 1)
